# revision 6
# baseline (speedup 1.0000x reference)
"""Gaussian duration-attention upsampler on 8 Trainium2 NeuronCores (v2).

out[b,t,:] = (sum_i w[b,i,t] * emb[b,i,:]) / (sum_i w[b,i,t] + eps) + PE[t,:]
  with w[b,i,t] = exp(-(t - c[b,i])^2 / ranges[b,i]^2), c = cumsum(dur) - dur/2.

v2 strategy (vs the 77us v1):
  - Data-parallel over batch: 4 batches/core, SPMD, no collectives.
  - Narrow Gaussians: per 128-frame output chunk only <=31 tokens matter
    (measured span max 30 on this data).  KW=32-token windows, CW=128-frame
    chunks, NJ=32 chunks/batch.  FOUR windows pack into the 128 partitions
    (partition 32k+i = token i of window k), so every op covers 4 windows.
  - sq = a*(t-c)^2 is computed on TensorE as a K=4 fp32 matmul per pack:
    lhsT = per-token quadratic coefs {a, -2ac'', ac''^2}, rhs = static
    {t''^2, t'', 1} (t'' chunk-centered so fp32 cancellation error ~2e-3).
    This removes ALL Square work from ScalarE (the v1 bottleneck); ScalarE
    keeps only the Exp (PSUM sq -> bf16 W) and un-scaled cast-copies.
  - Per window one K=32 matmul [W^T][eg|1] (N=257) into its own PSUM bank
    (hardware: one matmul accumulation region per bank - two matmuls into
    one bank hang the device).  4 windows/pack -> row-tiled
    tile_position=(32k,0), concurrent on the PE array.  The ones column
    yields the normalizer s in column 256; eps enters as a sacrificial
    token row with sq=-ln(eps) and zero embedding, so s already includes
    eps and 1/(s+eps) is a plain VectorE reciprocal off PSUM.
  - Postproc split to balance engines: most packs get a ScalarE batched
    cast-copy (PSUM U -> SBUF bf16, no per-window params) followed by
    VectorE scalar_tensor_tensor out = (U*r) + PE in 2x bf16 mode (194ns);
    every VEC_EVERY-th pack runs the stt directly from PSUM f32 (1x) on
    VectorE to soak V-slack.
  - Output frames need no permutation: chunk frame f sits on partition f;
    a pack's staging tile is [128, 4, 256] -> 4x512B DRAM lines/partition,
    one 256KB DMA per pack on the GpSimd SWDGE queue.
"""

from collections import deque

import numpy as np
import ml_dtypes

import concourse.bacc as bacc
import concourse.mybir as mybir
import concourse.tile as tile
from concourse.bass_utils import run_bass_kernel_spmd

BF16 = ml_dtypes.bfloat16

B, T_IN, D, T_OUT = 32, 512, 256, 4096
EPS = 1e-6
N_CORES = 8
BL = B // N_CORES          # batches per core (4)
CW = 128                   # chunk width (frames)
NJ = T_OUT // CW           # chunks per batch (32)
KW = 32                    # window tokens per chunk
NPACK = BL * NJ // 4       # packs per core (32); pack = (b, 4 consecutive j)
TH = 30.0                  # exp(-30) ~ 1e-13 banding threshold
NE = D + 1                 # eg columns (256 embedding + ones)

F32 = mybir.dt.float32
BF = mybir.dt.bfloat16

VEC_EVERY = 6              # every 6th pack takes the direct-PSUM VectorE path

_CACHE = {}


def _pe_table():
    pos = np.arange(T_OUT, dtype=np.float32)[:, None]
    div = np.exp(np.arange(0, D, 2, dtype=np.float32) * (-np.log(10000.0) / D))
    pe = np.zeros((T_OUT, D), np.float32)
    pe[:, 0::2] = np.sin(pos * div)
    pe[:, 1::2] = np.cos(pos * div)
    return pe


def _build():
    nc = bacc.Bacc(
        "TRN2",
        target_bir_lowering=False,
        debug=False,
        enable_asserts=False,
        num_devices=N_CORES,
    )
    eg_d = nc.dram_tensor("eg", (128, NPACK * NE), BF, kind="ExternalInput")
    par_d = nc.dram_tensor("par", (4, NPACK * 128), F32, kind="ExternalInput")
    quad_d = nc.dram_tensor("quad", (4, 128), F32, kind="ExternalInput")
    pe_d = nc.dram_tensor("pe", (128, NJ * D), BF, kind="ExternalInput")
    out_d = nc.dram_tensor("out", (BL, T_OUT, D), BF, kind="ExternalOutput")
    # frame 512*J + 128*k + t lives on partition t, free offset k*D + d
    outv = out_d[:].rearrange("b (J k t) d -> b J t k d", J=NJ // 4, k=4, t=128)

    Ex = mybir.ActivationFunctionType.Exp
    Cp = mybir.ActivationFunctionType.Copy
    ADD = mybir.AluOpType.add
    MUL = mybir.AluOpType.mult

    with tile.TileContext(nc) as tc:
        with (
            tc.tile_pool(name="const", bufs=1) as cp,
            tc.tile_pool(name="w", bufs=3) as wp,
            tc.tile_pool(name="ub", bufs=4) as ubp,
            tc.tile_pool(name="ob", bufs=6) as obp,
            tc.tile_pool(name="r", bufs=8) as rp,
            tc.tile_pool(name="psq", bufs=2, space="PSUM") as psqp,
            tc.tile_pool(name="pu", bufs=3, space="PSUM") as pup,
        ):
            # dummy activation with no DMA deps: pulls the Exp ACT-table load
            # to the head of the Scalar queue, overlapping it with input DMAs
            dmy = cp.tile([128, 8], F32)
            nc.gpsimd.memset(dmy[:], 0.0)
            zb = dmy[:, 0:1]
            dmy2 = cp.tile([128, 8], F32)
            nc.scalar.activation(dmy2[:], dmy[:], Ex, bias=zb, scale=-1.0)

            par_sb = cp.tile([4, NPACK * 128], F32)
            quad_sb = cp.tile([4, 128], F32)
            eg_sbs = [cp.tile([128, 8 * NE], BF, name=f"eg{b}") for b in range(BL)]
            pe_sbs = [cp.tile([128, 8 * D], BF, name=f"pe{q}") for q in range(4)]
            nc.sync.dma_start(par_sb[:], par_d[:])
            nc.sync.dma_start(quad_sb[:], quad_d[:])
            nc.sync.dma_start(eg_sbs[0][:], eg_d[:, 0 : 8 * NE])
            nc.sync.dma_start(pe_sbs[0][:], pe_d[:, 0 : 8 * D])
            nc.sync.dma_start(pe_sbs[1][:], pe_d[:, 8 * D : 16 * D])
            nc.sync.dma_start(eg_sbs[1][:], eg_d[:, 8 * NE : 16 * NE])
            nc.sync.dma_start(pe_sbs[2][:], pe_d[:, 16 * D : 24 * D])
            nc.sync.dma_start(pe_sbs[3][:], pe_d[:, 24 * D : 32 * D])
            nc.sync.dma_start(eg_sbs[2][:], eg_d[:, 16 * NE : 24 * NE])
            nc.sync.dma_start(eg_sbs[3][:], eg_d[:, 24 * NE : 32 * NE])

            obs = {}

            def emit_post(st):
                p, hp, ups, r2 = st
                b, pp = divmod(p, NJ // 4)
                if hp == 0:
                    obs[p] = obp.tile([128, 4, D], BF, name=f"ob{p}", tag="ob")
                ob = obs[p]
                pe_t = pe_sbs[pp // 2]
                po = (pp % 2) * 4 * D
                if p % VEC_EVERY == 0:
                    for kk in range(2):
                        k = 2 * hp + kk
                        nc.vector.scalar_tensor_tensor(
                            ob[:, k, :],
                            ups[:, kk, 0:D],
                            r2[:, kk : kk + 1],
                            pe_t[:, po + k * D : po + (k + 1) * D],
                            MUL,
                            ADD,
                        )
                else:
                    ub = ubp.tile([128, 2, D], BF, name=f"ub{p}_{hp}", tag="ub")
                    nc.scalar.activation(ub[:], ups[:, :, 0:D], Cp, bias=0.0)
                    for kk in range(2):
                        k = 2 * hp + kk
                        nc.vector.scalar_tensor_tensor(
                            ob[:, k, :],
                            ub[:, kk, :],
                            r2[:, kk : kk + 1],
                            pe_t[:, po + k * D : po + (k + 1) * D],
                            MUL,
                            ADD,
                        )
                if hp == 1:
                    nc.gpsimd.dma_start(outv[b, pp], ob[:])
                    del obs[p]

            pending = deque()
            for p in range(NPACK):
                b, pp = divmod(p, NJ // 4)
                sq = psqp.tile([128, 512], F32, name=f"sq{p}", tag="sq")
                nc.tensor.matmul(
                    sq[:, 0:128],
                    par_sb[0:4, p * 128 : (p + 1) * 128],
                    quad_sb[:],
                    start=True,
                    stop=True,
                )
                wg = wp.tile([128, 128], BF, name=f"w{p}", tag="w")
                nc.scalar.activation(wg[:], sq[:, 0:128], Ex, bias=zb, scale=-1.0)
                for hp in range(2):
                    ups = pup.tile([128, 2, 512], F32, name=f"u{p}_{hp}", tag="u")
                    for kk in range(2):
                        k = 2 * hp + kk
                        nc.tensor.matmul(
                            ups[:, kk, 0:NE],
                            wg[32 * k : 32 * k + 32, :],
                            eg_sbs[b][32 * k : 32 * k + 32, pp * NE : (pp + 1) * NE],
                            start=True,
                            stop=True,
                            tile_position=(32 * k, 0),
                        )
                    r2 = rp.tile([128, 2], F32, name=f"r{p}_{hp}", tag="r")
                    nc.vector.reciprocal(r2[:], ups[:, :, 256])
                    pending.append((p, hp, ups, r2))
                    while len(pending) > 2:
                        emit_post(pending.popleft())
            while pending:
                emit_post(pending.popleft())

    nc.compile()
    return nc


def kernel(embeddings, durations, ranges, t_out):
    assert int(t_out) == T_OUT
    emb = np.asarray(embeddings, dtype=np.float32)
    dur = np.asarray(durations, dtype=np.float32)[:, :, 0]
    rng = np.asarray(ranges, dtype=np.float32)[:, :, 0]

    # ---- host preprocessing: O(B*T_in) scalars + window selection ----
    c = np.cumsum(dur, axis=1, dtype=np.float32) - 0.5 * dur   # (B, T_IN)
    a = rng.astype(np.float32) ** -2
    reach = np.sqrt(TH) * rng

    lo_r, hi_r = c - reach, c + reach
    starts = np.zeros((B, NJ), np.int32)
    for b in range(B):
        for j in range(NJ):
            qual = np.nonzero((lo_r[b] <= CW * j + CW - 1) & (hi_r[b] >= CW * j))[0]
            if len(qual):
                assert qual[-1] - qual[0] + 1 <= KW - 1, "window overflow"
                starts[b, j] = qual[0]
    starts = np.minimum(starts, T_IN - (KW - 1))
    # coverage assert (windows are contiguous token ranges)
    for b in range(B):
        for j in range(NJ):
            qual = np.nonzero((lo_r[b] <= CW * j + CW - 1) & (hi_r[b] >= CW * j))[0]
            if len(qual):
                assert starts[b, j] <= qual[0] and qual[-1] < starts[b, j] + KW - 1

    kidx = starts[:, :, None] + np.arange(KW)[None, None, :]   # (B, NJ, KW)
    kidx = np.minimum(kidx, T_IN - 1)
    bidx = np.arange(B)[:, None, None]
    cg = c[bidx, kidx]
    ag = a[bidx, kidx]
    center = (np.arange(NJ, dtype=np.float32) * CW + CW / 2)[None, :, None]
    cc = cg - center
    co = np.zeros((4, B, NJ, KW), np.float32)
    co[0] = ag
    co[1] = -2.0 * ag * cc
    co[2] = ag * cc * cc
    # sacrificial eps token: sq = -ln(eps) const -> w = eps for all t
    co[0, :, :, KW - 1] = 0.0
    co[1, :, :, KW - 1] = 0.0
    co[2, :, :, KW - 1] = np.float32(-np.log(EPS))

    egg = np.ones((B, NJ, KW, NE), BF16)
    egg[:, :, :, 0:D] = emb[bidx, kidx].astype(BF16)            # (B, NJ, KW, D)
    egg[:, :, KW - 1, 0:D] = 0

    f = np.arange(128, dtype=np.float32)
    quad = np.stack([(f - 64.0) ** 2, f - 64.0, np.ones(128, np.float32),
                     np.zeros(128, np.float32)], 0)             # (4, 128)
    pe = _pe_table().reshape(NJ, 128, D).transpose(1, 0, 2).reshape(128, NJ * D)
    pe = pe.astype(BF16)

    if 0 not in _CACHE:
        _CACHE[0] = _build()
    nc = _CACHE[0]

    in_maps = []
    for i in range(N_CORES):
        bs = slice(i * BL, (i + 1) * BL)
        # eg: partition 32k+i <- token i of window k; col (b*8+pp)*NE + e
        eg5 = egg[bs].reshape(BL, NJ // 4, 4, KW, NE)           # (b, pp, k, i, e)
        eg_core = np.ascontiguousarray(
            eg5.transpose(2, 3, 0, 1, 4).reshape(4 * KW, NPACK * NE)
        )
        co5 = co[:, bs].reshape(4, BL, NJ // 4, 4, KW)          # (r, b, pp, k, i)
        par_core = np.ascontiguousarray(co5.reshape(4, NPACK * 128))
        in_maps.append({
            "eg": eg_core,
            "par": par_core,
            "quad": quad,
            "pe": pe,
        })

    res = run_bass_kernel_spmd(nc, in_maps, core_ids=list(range(N_CORES)))
    out = np.concatenate([r["out"] for r in res.results], axis=0)
    return out.astype(np.float32)


# revision 8
# speedup vs baseline: 1.0920x; 1.0920x over previous
"""Gaussian duration-attention upsampler on 8 Trainium2 NeuronCores (v2).

out[b,t,:] = (sum_i w[b,i,t] * emb[b,i,:]) / (sum_i w[b,i,t] + eps) + PE[t,:]
  with w[b,i,t] = exp(-(t - c[b,i])^2 / ranges[b,i]^2), c = cumsum(dur) - dur/2.

v2 strategy (vs the 77us v1):
  - Data-parallel over batch: 4 batches/core, SPMD, no collectives.
  - Narrow Gaussians: per 128-frame output chunk only <=31 tokens matter
    (measured span max 30 on this data).  KW=32-token windows, CW=128-frame
    chunks, NJ=32 chunks/batch.  FOUR windows pack into the 128 partitions
    (partition 32k+i = token i of window k), so every op covers 4 windows.
  - sq = a*(t-c)^2 is computed on TensorE as a K=4 fp32 matmul per pack:
    lhsT = per-token quadratic coefs {a, -2ac'', ac''^2}, rhs = static
    {t''^2, t'', 1} (t'' chunk-centered so fp32 cancellation error ~2e-3).
    This removes ALL Square work from ScalarE (the v1 bottleneck); ScalarE
    keeps only the Exp (PSUM sq -> bf16 W) and un-scaled cast-copies.
  - Per window one K=32 matmul [W^T][eg|1] (N=257) into its own PSUM bank
    (hardware: one matmul accumulation region per bank - two matmuls into
    one bank hang the device).  4 windows/pack -> row-tiled
    tile_position=(32k,0), concurrent on the PE array.  The ones column
    yields the normalizer s in column 256; eps enters as a sacrificial
    token row with sq=-ln(eps) and zero embedding, so s already includes
    eps and 1/(s+eps) is a plain VectorE reciprocal off PSUM.
  - Postproc split to balance engines: most packs get a ScalarE batched
    cast-copy (PSUM U -> SBUF bf16, no per-window params) followed by
    VectorE scalar_tensor_tensor out = (U*r) + PE in 2x bf16 mode (194ns);
    every VEC_EVERY-th pack runs the stt directly from PSUM f32 (1x) on
    VectorE to soak V-slack.
  - Output frames need no permutation: chunk frame f sits on partition f;
    a pack's staging tile is [128, 4, 256] -> 4x512B DRAM lines/partition,
    one 256KB DMA per pack on the GpSimd SWDGE queue.
"""

from collections import deque

import numpy as np
import ml_dtypes

import concourse.bacc as bacc
import concourse.mybir as mybir
import concourse.tile as tile
from concourse.bass_utils import run_bass_kernel_spmd

BF16 = ml_dtypes.bfloat16

B, T_IN, D, T_OUT = 32, 512, 256, 4096
EPS = 1e-6
N_CORES = 8
BL = B // N_CORES          # batches per core (4)
CW = 128                   # chunk width (frames)
NJ = T_OUT // CW           # chunks per batch (32)
KW = 32                    # window tokens per chunk
NPACK = BL * NJ // 4       # packs per core (32); pack = (b, 4 consecutive j)
TH = 30.0                  # exp(-30) ~ 1e-13 banding threshold
NE = D + 1                 # eg columns (256 embedding + ones)

F32 = mybir.dt.float32
BF = mybir.dt.bfloat16

N_WARM = 16                # dummy matmuls to un-throttle the PE HAM clock gate

_CACHE = {}


def _pe_table():
    pos = np.arange(T_OUT, dtype=np.float32)[:, None]
    div = np.exp(np.arange(0, D, 2, dtype=np.float32) * (-np.log(10000.0) / D))
    pe = np.zeros((T_OUT, D), np.float32)
    pe[:, 0::2] = np.sin(pos * div)
    pe[:, 1::2] = np.cos(pos * div)
    return pe


def _build():
    nc = bacc.Bacc(
        "TRN2",
        target_bir_lowering=False,
        debug=False,
        enable_asserts=False,
        num_devices=N_CORES,
    )
    eg_d = nc.dram_tensor("eg", (128, NPACK * NE), BF, kind="ExternalInput")
    par_d = nc.dram_tensor("par", (4, NPACK * 128), F32, kind="ExternalInput")
    quad_d = nc.dram_tensor("quad", (4, 128), F32, kind="ExternalInput")
    pe_d = nc.dram_tensor("pe", (128, NJ * D), BF, kind="ExternalInput")
    out_d = nc.dram_tensor("out", (BL, T_OUT, D), BF, kind="ExternalOutput")
    # frame 512*J + 128*k + t lives on partition t, free offset k*D + d
    outv = out_d[:].rearrange("b (J k t) d -> b J t k d", J=NJ // 4, k=4, t=128)

    Ex = mybir.ActivationFunctionType.Exp
    Iden = mybir.ActivationFunctionType.Identity
    ADD = mybir.AluOpType.add
    MUL = mybir.AluOpType.mult

    with tile.TileContext(nc) as tc:
        with (
            tc.tile_pool(name="const", bufs=1) as cp,
            tc.tile_pool(name="w", bufs=3) as wp,
            tc.tile_pool(name="ub", bufs=3) as ubp,
            tc.tile_pool(name="ob", bufs=6) as obp,
            tc.tile_pool(name="r", bufs=8) as rp,
            tc.tile_pool(name="psq", bufs=1, space="PSUM") as psqp,
            tc.tile_pool(name="pu", bufs=3, space="PSUM") as pup,
        ):
            # dummy activation with no DMA deps: pulls the Exp ACT-table load
            # to the head of the Scalar queue, overlapping it with input DMAs
            dmy = cp.tile([128, 8], F32)
            nc.gpsimd.memset(dmy[:], 0.0)
            zb = dmy[:, 0:1]
            dmy2 = cp.tile([128, 8], F32)
            nc.scalar.activation(dmy2[:], dmy[:], Ex, bias=zb, scale=-1.0)

            # HAM warm-up: a burst of dummy back-to-back matmuls (no DMA deps)
            # runs during the input DMAs and un-throttles the PE clock gate
            # (cold 1.2GHz K=4/8 -> warm 2.4GHz K=8/8) before the real matmuls
            wdm = cp.tile([128, 128], BF)
            edm = cp.tile([128, 512], BF)
            nc.gpsimd.memset(wdm[:], 0.25)
            nc.gpsimd.memset(edm[:], 0.25)
            for i in range(N_WARM):
                wps = pup.tile([128, 2, 512], F32, name=f"warm{i}", tag="u")
                nc.tensor.matmul(wps[:, 0, :], wdm[:], edm[:], start=True, stop=True)

            par_sb = cp.tile([4, NPACK * 128], F32)
            quad_sb = cp.tile([4, 128], F32)
            eg_sbs = [cp.tile([128, 8 * NE], BF, name=f"eg{b}") for b in range(BL)]
            pe_sbs = [cp.tile([128, 8 * D], BF, name=f"pe{q}") for q in range(4)]
            nc.sync.dma_start(par_sb[:], par_d[:])
            nc.sync.dma_start(quad_sb[:], quad_d[:])
            nc.sync.dma_start(eg_sbs[0][:], eg_d[:, 0 : 8 * NE])
            nc.sync.dma_start(pe_sbs[0][:], pe_d[:, 0 : 8 * D])
            nc.sync.dma_start(pe_sbs[1][:], pe_d[:, 8 * D : 16 * D])
            nc.sync.dma_start(eg_sbs[1][:], eg_d[:, 8 * NE : 16 * NE])
            nc.sync.dma_start(pe_sbs[2][:], pe_d[:, 16 * D : 24 * D])
            nc.sync.dma_start(pe_sbs[3][:], pe_d[:, 24 * D : 32 * D])
            nc.sync.dma_start(eg_sbs[2][:], eg_d[:, 16 * NE : 24 * NE])
            nc.sync.dma_start(eg_sbs[3][:], eg_d[:, 24 * NE : 32 * NE])

            obs = {}
            ubs = {}

            def emit_post(st):
                p, hp, ups, r2 = st
                b, pp = divmod(p, NJ // 4)
                pe_t = pe_sbs[pp // 2]
                po = (pp % 2) * 4 * D
                if p % 2 == 0:
                    # V-direct path: stt straight off PSUM does cast +
                    # normalize + PE-add in one 1x pass per window
                    if hp == 0:
                        obs[p] = obp.tile([128, 4 * D], BF, name=f"ob{p}", tag="ob")
                    ob = obs[p]
                    for kk in range(2):
                        k = 2 * hp + kk
                        nc.vector.scalar_tensor_tensor(
                            ob[:, k * D : (k + 1) * D],
                            ups[:, kk, 0:D],
                            r2[:, kk : kk + 1],
                            pe_t[:, po + k * D : po + (k + 1) * D],
                            MUL,
                            ADD,
                        )
                else:
                    # S path: per-window scaled cast-copy on ScalarE, then one
                    # whole-pack bf16 PE-add on VectorE (2x) or GpSimd
                    if hp == 0:
                        ubs[p] = ubp.tile([128, 4 * D], BF, name=f"ub{p}", tag="ub")
                    ub = ubs[p]
                    for kk in range(2):
                        k = 2 * hp + kk
                        nc.scalar.activation(
                            ub[:, k * D : (k + 1) * D],
                            ups[:, kk, 0:D],
                            Iden,
                            bias=zb,
                            scale=r2[:, kk : kk + 1],
                        )
                    if hp == 1:
                        obs[p] = obp.tile([128, 4 * D], BF, name=f"ob{p}", tag="ob")
                        eng = nc.gpsimd if p % 4 == 3 else nc.vector
                        eng.tensor_tensor(
                            obs[p][:], ub[:], pe_t[:, po : po + 4 * D], ADD
                        )
                        del ubs[p]
                if hp == 1:
                    ob = obs[p]
                    nc.sync.dma_start(
                        outv[b, pp], ob[:].rearrange("t (k d) -> t k d", k=4)
                    )
                    del obs[p]

            pending = deque()
            for g in range(NPACK // 2):
                sqpair = psqp.tile([128, 2, 512], F32, name=f"sq{g}", tag="sq")
                for i in range(2):
                    p = 2 * g + i
                    nc.tensor.matmul(
                        sqpair[:, i, 0:128],
                        par_sb[0:4, p * 128 : (p + 1) * 128],
                        quad_sb[:],
                        start=True,
                        stop=True,
                    )
                wgp = wp.tile([128, 2, 128], BF, name=f"w{g}", tag="w")
                nc.scalar.activation(
                    wgp[:], sqpair[:, :, 0:128], Ex, bias=zb, scale=-1.0
                )
                for i in range(2):
                    p = 2 * g + i
                    b, pp = divmod(p, NJ // 4)
                    for hp in range(2):
                        ups = pup.tile([128, 2, 512], F32, name=f"u{p}_{hp}", tag="u")
                        for kk in range(2):
                            k = 2 * hp + kk
                            nc.tensor.matmul(
                                ups[:, kk, 0:NE],
                                wgp[32 * k : 32 * k + 32, i, :],
                                eg_sbs[b][32 * k : 32 * k + 32, pp * NE : (pp + 1) * NE],
                                start=True,
                                stop=True,
                                tile_position=(32 * k, 0),
                            )
                        r2 = rp.tile([128, 2], F32, name=f"r{p}_{hp}", tag="r")
                        nc.vector.reciprocal(r2[:], ups[:, :, 256])
                        pending.append((p, hp, ups, r2))
                        while len(pending) > 2:
                            emit_post(pending.popleft())
            while pending:
                emit_post(pending.popleft())

    nc.compile()
    return nc


def kernel(embeddings, durations, ranges, t_out):
    assert int(t_out) == T_OUT
    emb = np.asarray(embeddings, dtype=np.float32)
    dur = np.asarray(durations, dtype=np.float32)[:, :, 0]
    rng = np.asarray(ranges, dtype=np.float32)[:, :, 0]

    # ---- host preprocessing: O(B*T_in) scalars + window selection ----
    c = np.cumsum(dur, axis=1, dtype=np.float32) - 0.5 * dur   # (B, T_IN)
    a = rng.astype(np.float32) ** -2
    reach = np.sqrt(TH) * rng

    lo_r, hi_r = c - reach, c + reach
    starts = np.zeros((B, NJ), np.int32)
    for b in range(B):
        for j in range(NJ):
            qual = np.nonzero((lo_r[b] <= CW * j + CW - 1) & (hi_r[b] >= CW * j))[0]
            if len(qual):
                assert qual[-1] - qual[0] + 1 <= KW - 1, "window overflow"
                starts[b, j] = qual[0]
    starts = np.minimum(starts, T_IN - (KW - 1))
    # coverage assert (windows are contiguous token ranges)
    for b in range(B):
        for j in range(NJ):
            qual = np.nonzero((lo_r[b] <= CW * j + CW - 1) & (hi_r[b] >= CW * j))[0]
            if len(qual):
                assert starts[b, j] <= qual[0] and qual[-1] < starts[b, j] + KW - 1

    kidx = starts[:, :, None] + np.arange(KW)[None, None, :]   # (B, NJ, KW)
    kidx = np.minimum(kidx, T_IN - 1)
    bidx = np.arange(B)[:, None, None]
    cg = c[bidx, kidx]
    ag = a[bidx, kidx]
    center = (np.arange(NJ, dtype=np.float32) * CW + CW / 2)[None, :, None]
    cc = cg - center
    co = np.zeros((4, B, NJ, KW), np.float32)
    co[0] = ag
    co[1] = -2.0 * ag * cc
    co[2] = ag * cc * cc
    # sacrificial eps token: sq = -ln(eps) const -> w = eps for all t
    co[0, :, :, KW - 1] = 0.0
    co[1, :, :, KW - 1] = 0.0
    co[2, :, :, KW - 1] = np.float32(-np.log(EPS))

    egg = np.ones((B, NJ, KW, NE), BF16)
    egg[:, :, :, 0:D] = emb[bidx, kidx].astype(BF16)            # (B, NJ, KW, D)
    egg[:, :, KW - 1, 0:D] = 0

    f = np.arange(128, dtype=np.float32)
    quad = np.stack([(f - 64.0) ** 2, f - 64.0, np.ones(128, np.float32),
                     np.zeros(128, np.float32)], 0)             # (4, 128)
    pe = _pe_table().reshape(NJ, 128, D).transpose(1, 0, 2).reshape(128, NJ * D)
    pe = pe.astype(BF16)

    if 0 not in _CACHE:
        _CACHE[0] = _build()
    nc = _CACHE[0]

    in_maps = []
    for i in range(N_CORES):
        bs = slice(i * BL, (i + 1) * BL)
        # eg: partition 32k+i <- token i of window k; col (b*8+pp)*NE + e
        eg5 = egg[bs].reshape(BL, NJ // 4, 4, KW, NE)           # (b, pp, k, i, e)
        eg_core = np.ascontiguousarray(
            eg5.transpose(2, 3, 0, 1, 4).reshape(4 * KW, NPACK * NE)
        )
        co5 = co[:, bs].reshape(4, BL, NJ // 4, 4, KW)          # (r, b, pp, k, i)
        par_core = np.ascontiguousarray(co5.reshape(4, NPACK * 128))
        in_maps.append({
            "eg": eg_core,
            "par": par_core,
            "quad": quad,
            "pe": pe,
        })

    res = run_bass_kernel_spmd(nc, in_maps, core_ids=list(range(N_CORES)))
    out = np.concatenate([r["out"] for r in res.results], axis=0)
    return out.astype(np.float32)


# revision 9
# speedup vs baseline: 1.2917x; 1.1829x over previous
"""Gaussian duration-attention upsampler on 8 Trainium2 NeuronCores (v4).

out[b,t,:] = (sum_i w[b,i,t] * emb[b,i,:]) / (sum_i w[b,i,t] + eps) + PE[t,:]
  with w[b,i,t] = exp(-(t - c[b,i])^2 / ranges[b,i]^2), c = cumsum(dur) - dur/2.

Strategy:
  - Data-parallel over batch: 4 batches/core, SPMD, no collectives.
  - Narrow Gaussians: per 128-frame output chunk only <=31 tokens matter
    (measured span max 30 on this data).  KW=32-token windows, CW=128-frame
    chunks, NJ=32 chunks/batch.  FOUR windows pack into the 128 partitions
    (partition 32k+i = token i of window k), so every W-gen op covers 4
    windows: W-gen element count is 4x smaller than 128-token banding.
  - W-gen on ScalarE: per pack one Square (per-partition scale/bias:
    sq = (sqa*t'' - sqa*c'')^2, FD=128) + per pack-pair one Exp
    (FD=256, SBUF).  All W-gen lives in SBUF and is emitted TWO pack-pairs
    ahead of the consuming matmuls so LDWEIGHTS never stalls the PE.
  - Per window one K=32 matmul [W^T][eg|1] (N=257) into its own PSUM bank
    (hardware: one matmul accumulation region per bank - two matmuls into
    one bank hang the device).  4 windows/pack at row-tiled
    tile_position=(32k,0).  The ones column yields the normalizer s in
    column 256; eps enters as a sacrificial token row with sq=-ln(eps)
    and zero embedding, so 1/(s+eps) is a plain VectorE reciprocal.
    PSUM holds nothing else: U tiles get all 8 banks (bufs=4 half-pack
    tiles), postprocessing lags 3 half-packs behind the matmuls.
  - A 16-matmul dummy burst at t=0 (during input DMAs) pushes the PE HAM
    clock gate to warm before real matmuls begin.
  - Postproc split to balance S and V: V-packs run scalar_tensor_tensor
    straight off PSUM (cast+normalize+PE-add in one 1x pass per window);
    S-packs run per-window scaled cast-copies on ScalarE (Identity with
    per-partition scale=r) plus one whole-pack bf16 PE-add (VectorE 2x
    tensor_tensor, or GpSimd for a few packs to soak idle cycles).
  - Output frames need no permutation: chunk frame f sits on partition f;
    a pack's staging tile is [128, 4x256] -> 4x512B DRAM lines/partition,
    one 256KB DMA per pack, alternating between the Sync HWDGE queue and
    the GpSimd SWDGE queue.
"""

from collections import deque

import numpy as np
import ml_dtypes

import concourse.bacc as bacc
import concourse.mybir as mybir
import concourse.tile as tile
from concourse.bass_utils import run_bass_kernel_spmd

BF16 = ml_dtypes.bfloat16

B, T_IN, D, T_OUT = 32, 512, 256, 4096
EPS = 1e-6
N_CORES = 8
BL = B // N_CORES          # batches per core (4)
CW = 128                   # chunk width (frames)
NJ = T_OUT // CW           # chunks per batch (32)
KW = 32                    # window tokens per chunk
NPACK = BL * NJ // 4       # packs per core (32); pack = (b, 4 consecutive j)
TH = 30.0                  # exp(-30) ~ 1e-13 banding threshold
NE = D + 1                 # eg columns (256 embedding + ones)

F32 = mybir.dt.float32
BF = mybir.dt.bfloat16

N_WARM = 16                # dummy matmuls to un-throttle the PE HAM clock gate
WAHEAD = 2                 # pack-pairs of W generated ahead of their matmuls
LAG = 3                    # half-packs between matmuls and postprocessing
# packs whose postproc goes ScalarE-copy + tensor_tensor PE-add (rest: V stt)
S_PACKS = frozenset(round(i * 32 / 14) for i in range(14))
G_PACKS = frozenset(list(sorted(S_PACKS))[::2][:6])   # their PE-add on GpSimd

_CACHE = {}


def _pe_table():
    pos = np.arange(T_OUT, dtype=np.float32)[:, None]
    div = np.exp(np.arange(0, D, 2, dtype=np.float32) * (-np.log(10000.0) / D))
    pe = np.zeros((T_OUT, D), np.float32)
    pe[:, 0::2] = np.sin(pos * div)
    pe[:, 1::2] = np.cos(pos * div)
    return pe


def _build():
    nc = bacc.Bacc(
        "TRN2",
        target_bir_lowering=False,
        debug=False,
        enable_asserts=False,
        num_devices=N_CORES,
    )
    eg_d = nc.dram_tensor("eg", (128, NPACK * NE), BF, kind="ExternalInput")
    par_d = nc.dram_tensor("par", (128, 2 * NPACK + 128), F32, kind="ExternalInput")
    pe_d = nc.dram_tensor("pe", (128, NJ * D), BF, kind="ExternalInput")
    out_d = nc.dram_tensor("out", (BL, T_OUT, D), BF, kind="ExternalOutput")
    # frame 512*J + 128*k + t lives on partition t, free offset k*D + d
    outv = out_d[:].rearrange("b (J k t) d -> b J t k d", J=NJ // 4, k=4, t=128)

    Sq = mybir.ActivationFunctionType.Square
    Ex = mybir.ActivationFunctionType.Exp
    Iden = mybir.ActivationFunctionType.Identity
    ADD = mybir.AluOpType.add
    MUL = mybir.AluOpType.mult

    with tile.TileContext(nc) as tc:
        with (
            tc.tile_pool(name="const", bufs=1) as cp,
            tc.tile_pool(name="sq", bufs=3) as sqp,
            tc.tile_pool(name="w", bufs=4) as wp,
            tc.tile_pool(name="ub", bufs=3) as ubp,
            tc.tile_pool(name="ob", bufs=8) as obp,
            tc.tile_pool(name="r", bufs=8) as rp,
            tc.tile_pool(name="pu", bufs=4, space="PSUM") as pup,
        ):
            # dummy activation with no DMA deps: pulls the ACT-table load
            # to the head of the Scalar queue, overlapping it with input DMAs
            dmy = cp.tile([128, 8], F32)
            nc.gpsimd.memset(dmy[:], 0.0)
            zb = dmy[:, 0:1]
            dmy2 = cp.tile([128, 8], F32)
            nc.scalar.activation(dmy2[:], dmy[:], Sq, bias=zb)
            nc.scalar.activation(dmy2[:], dmy[:], Ex, bias=zb, scale=-1.0)

            # HAM warm-up: a burst of dummy back-to-back matmuls (no DMA deps)
            # runs during the input DMAs and un-throttles the PE clock gate
            wdm = cp.tile([128, 128], BF)
            edm = cp.tile([128, 512], BF)
            nc.gpsimd.memset(wdm[:], 0.25)
            nc.gpsimd.memset(edm[:], 0.25)
            for i in range(N_WARM):
                wps = pup.tile([128, 2, 512], F32, name=f"warm{i}", tag="u")
                nc.tensor.matmul(wps[:, 0, :], wdm[:], edm[:], start=True, stop=True)

            # par: [sqa (NPACK) | nsqac (NPACK) | iota (128)] per partition
            par_sb = cp.tile([128, 2 * NPACK + 128], F32)
            eg_sbs = [cp.tile([128, 8 * NE], BF, name=f"eg{b}") for b in range(BL)]
            pe_sbs = [cp.tile([128, 8 * D], BF, name=f"pe{q}") for q in range(4)]
            nc.sync.dma_start(par_sb[:], par_d[:])
            nc.sync.dma_start(eg_sbs[0][:], eg_d[:, 0 : 8 * NE])
            nc.sync.dma_start(pe_sbs[0][:], pe_d[:, 0 : 8 * D])
            nc.sync.dma_start(pe_sbs[1][:], pe_d[:, 8 * D : 16 * D])
            nc.sync.dma_start(eg_sbs[1][:], eg_d[:, 8 * NE : 16 * NE])
            nc.sync.dma_start(pe_sbs[2][:], pe_d[:, 16 * D : 24 * D])
            nc.sync.dma_start(pe_sbs[3][:], pe_d[:, 24 * D : 32 * D])
            nc.sync.dma_start(eg_sbs[2][:], eg_d[:, 16 * NE : 24 * NE])
            nc.sync.dma_start(eg_sbs[3][:], eg_d[:, 24 * NE : 32 * NE])
            sqa_sb = par_sb[:, 0:NPACK]
            nsq_sb = par_sb[:, NPACK : 2 * NPACK]
            iota_sb = par_sb[:, 2 * NPACK :]

            obs = {}
            ubs = {}

            def emit_wgen(g):
                sqt = sqp.tile([128, 2, 128], F32, name=f"sq{g}", tag="sq")
                for i in range(2):
                    p = 2 * g + i
                    nc.scalar.activation(
                        sqt[:, i, :], iota_sb, Sq,
                        bias=nsq_sb[:, p : p + 1],
                        scale=sqa_sb[:, p : p + 1],
                    )
                wgp = wp.tile([128, 2, 128], BF, name=f"w{g}", tag="w")
                nc.scalar.activation(wgp[:], sqt[:], Ex, bias=zb, scale=-1.0)
                return wgp

            def emit_post(st):
                p, hp, ups, r2 = st
                b, pp = divmod(p, NJ // 4)
                pe_t = pe_sbs[pp // 2]
                po = (pp % 2) * 4 * D
                if p not in S_PACKS:
                    # V-direct path: stt straight off PSUM does cast +
                    # normalize + PE-add in one 1x pass per window
                    if hp == 0:
                        obs[p] = obp.tile([128, 4 * D], BF, name=f"ob{p}", tag="ob")
                    ob = obs[p]
                    for kk in range(2):
                        k = 2 * hp + kk
                        nc.vector.scalar_tensor_tensor(
                            ob[:, k * D : (k + 1) * D],
                            ups[:, kk, 0:D],
                            r2[:, kk : kk + 1],
                            pe_t[:, po + k * D : po + (k + 1) * D],
                            MUL,
                            ADD,
                        )
                else:
                    # S path: per-window scaled cast-copy on ScalarE, then one
                    # whole-pack bf16 PE-add on VectorE (2x) or GpSimd
                    if hp == 0:
                        ubs[p] = ubp.tile([128, 4 * D], BF, name=f"ub{p}", tag="ub")
                    ub = ubs[p]
                    for kk in range(2):
                        k = 2 * hp + kk
                        nc.scalar.activation(
                            ub[:, k * D : (k + 1) * D],
                            ups[:, kk, 0:D],
                            Iden,
                            bias=zb,
                            scale=r2[:, kk : kk + 1],
                        )
                    if hp == 1:
                        obs[p] = obp.tile([128, 4 * D], BF, name=f"ob{p}", tag="ob")
                        eng = nc.gpsimd if p in G_PACKS else nc.vector
                        eng.tensor_tensor(
                            obs[p][:], ub[:], pe_t[:, po : po + 4 * D], ADD
                        )
                        del ubs[p]
                if hp == 1:
                    ob = obs[p]
                    eng = nc.sync if p % 2 == 0 else nc.gpsimd
                    eng.dma_start(
                        outv[b, pp], ob[:].rearrange("t (k d) -> t k d", k=4)
                    )
                    del obs[p]

            pending = deque()
            wgs = {}
            for g in range(WAHEAD):
                wgs[g] = emit_wgen(g)
            for g in range(NPACK // 2):
                if g + WAHEAD < NPACK // 2:
                    wgs[g + WAHEAD] = emit_wgen(g + WAHEAD)
                wgp = wgs.pop(g)
                for i in range(2):
                    p = 2 * g + i
                    b, pp = divmod(p, NJ // 4)
                    for hp in range(2):
                        ups = pup.tile([128, 2, 512], F32, name=f"u{p}_{hp}", tag="u")
                        for kk in range(2):
                            k = 2 * hp + kk
                            nc.tensor.matmul(
                                ups[:, kk, 0:NE],
                                wgp[32 * k : 32 * k + 32, i, :],
                                eg_sbs[b][32 * k : 32 * k + 32, pp * NE : (pp + 1) * NE],
                                start=True,
                                stop=True,
                                tile_position=(32 * k, 0),
                            )
                        r2 = rp.tile([128, 2], F32, name=f"r{p}_{hp}", tag="r")
                        nc.vector.reciprocal(r2[:], ups[:, :, 256])
                        pending.append((p, hp, ups, r2))
                        while len(pending) > LAG:
                            emit_post(pending.popleft())
            while pending:
                emit_post(pending.popleft())

    nc.compile()
    return nc


def kernel(embeddings, durations, ranges, t_out):
    assert int(t_out) == T_OUT
    emb = np.asarray(embeddings, dtype=np.float32)
    dur = np.asarray(durations, dtype=np.float32)[:, :, 0]
    rng = np.asarray(ranges, dtype=np.float32)[:, :, 0]

    # ---- host preprocessing: O(B*T_in) scalars + window selection ----
    c = np.cumsum(dur, axis=1, dtype=np.float32) - 0.5 * dur   # (B, T_IN)
    a = rng.astype(np.float32) ** -2
    reach = np.sqrt(TH) * rng

    lo_r, hi_r = c - reach, c + reach
    starts = np.zeros((B, NJ), np.int32)
    for b in range(B):
        for j in range(NJ):
            qual = np.nonzero((lo_r[b] <= CW * j + CW - 1) & (hi_r[b] >= CW * j))[0]
            if len(qual):
                assert qual[-1] - qual[0] + 1 <= KW - 1, "window overflow"
                starts[b, j] = qual[0]
    starts = np.minimum(starts, T_IN - (KW - 1))
    # coverage assert (windows are contiguous token ranges)
    for b in range(B):
        for j in range(NJ):
            qual = np.nonzero((lo_r[b] <= CW * j + CW - 1) & (hi_r[b] >= CW * j))[0]
            if len(qual):
                assert starts[b, j] <= qual[0] and qual[-1] < starts[b, j] + KW - 1

    kidx = starts[:, :, None] + np.arange(KW)[None, None, :]   # (B, NJ, KW)
    kidx = np.minimum(kidx, T_IN - 1)
    bidx = np.arange(B)[:, None, None]
    cg = c[bidx, kidx]
    ag = a[bidx, kidx]
    center = (np.arange(NJ, dtype=np.float32) * CW + CW / 2)[None, :, None]
    cc = cg - center
    sqa = np.sqrt(ag)
    nsq = -sqa * cc
    # sacrificial eps token: sq = -ln(eps) const -> w = eps for all t
    sqa[:, :, KW - 1] = 0.0
    nsq[:, :, KW - 1] = np.float32(np.sqrt(-np.log(EPS)))

    egg = np.ones((B, NJ, KW, NE), BF16)
    egg[:, :, :, 0:D] = emb[bidx, kidx].astype(BF16)            # (B, NJ, KW, D)
    egg[:, :, KW - 1, 0:D] = 0

    iota = np.broadcast_to(
        np.arange(128, dtype=np.float32) - 64.0, (128, 128)
    ).copy()
    pe = _pe_table().reshape(NJ, 128, D).transpose(1, 0, 2).reshape(128, NJ * D)
    pe = pe.astype(BF16)

    if 0 not in _CACHE:
        _CACHE[0] = _build()
    nc = _CACHE[0]

    in_maps = []
    for i in range(N_CORES):
        bs = slice(i * BL, (i + 1) * BL)
        # eg: partition 32k+i <- token i of window k; col (b*8+pp)*NE + e
        eg5 = egg[bs].reshape(BL, NJ // 4, 4, KW, NE)           # (b, pp, k, i, e)
        eg_core = np.ascontiguousarray(
            eg5.transpose(2, 3, 0, 1, 4).reshape(4 * KW, NPACK * NE)
        )
        # par: [sqa | nsqac | iota]; col p, partition 32k+i
        sqa5 = sqa[bs].reshape(BL, NJ // 4, 4, KW)
        sqa_core = sqa5.transpose(2, 3, 0, 1).reshape(4 * KW, NPACK)
        nsq5 = nsq[bs].reshape(BL, NJ // 4, 4, KW)
        nsq_core = nsq5.transpose(2, 3, 0, 1).reshape(4 * KW, NPACK)
        par_core = np.ascontiguousarray(
            np.concatenate([sqa_core, nsq_core, iota], axis=1).astype(np.float32)
        )
        in_maps.append({
            "eg": eg_core,
            "par": par_core,
            "pe": pe,
        })

    res = run_bass_kernel_spmd(nc, in_maps, core_ids=list(range(N_CORES)))
    out = np.concatenate([r["out"] for r in res.results], axis=0)
    return out.astype(np.float32)
